# revision 24
# baseline (speedup 1.0000x reference)
"""nn_AdapFilter3d Trainium2 kernel — 8-core SPMD (data-parallel over (B,C)).

out[b,c,z,y,x] = sum_{i,j,k} pad(input)[b,c,z+i-1,y+j-1,x+k-1] * F[b,c,z,y,x,i,j,k]

v6 strategy (per NeuronCore: 4 of the 32 (b,c) slices = 2 slice-pairs;
partitions p = 64*s + y; free dims carry (z, x) densely):

  - F streams from HBM as fp8 E3M4 (4 mantissa bits): halves the dominant
    HBM stream (28.3MB -> 14.2MB/core). End-to-end rel err 1.28e-2 < 2e-2.
  - y-shift via accumulating matmuls with shift stationaries S_j (host
    pre-shifts F by -dy per j); x/z shifts are free-dim offsets into one
    padded dense (z,x) row per y-partition.
  - Taps in 9 slots (3 i-taps each; j,k fixed per slot). 3 slots multiply
    fused fp8 x bf16 on DVE (1x mode, 557ns/tap), 6 slots are ScalarE-
    upconverted fp8->bf16 (427ns/tap) then DVE bf16 2x (278ns/tap).
    Keeps Scalar duty ~60% — pushing more work onto ScalarE was measured
    to trigger a chip-wide ~20% downclock (P0/activity throttle).
  - Merged multi-slot DVE instrs (stride-0 j-broadcast APs) hit 2x fine;
    k=1 windows have odd bases, served via a +1-shifted second x copy.
  - gpsimd computes NOTHING (measured 6.8x DVE slowdown from SBUF-fabric
    contention); it only carries the output-DMA descriptors.
  - PE: 27 accumulating 512-col matmuls/chunk (216ns warm / 427 cold).

Self-contained: hardcodes shapes from the problem spec.
"""

import time

import numpy as np

import bass_rust
import concourse.bacc as bacc
import concourse.tile as tile
from concourse import mybir
from concourse.bass_utils import run_bass_kernel_spmd

B, C, D, H, W = 2, 16, 32, 64, 64
BC = B * C
TAPS = 27
N_CORES = 8
S = BC // N_CORES  # 4 slices per core
PAIRS = S // 2  # 2
ZC = 8  # z planes per chunk
NCHUNK = D // ZC  # 4
FD = ZC * W  # 512
SFD = 3 * FD  # 1536 (one slot = 3 i-taps)
CFD = TAPS * FD  # 13824 (one chunk of F)
DW = D * W  # 2048 dense (z,x) elements per (slice, y)
FRONT = 65  # zero pad around the dense (z,x) block (>= W+1)
XPLEN = FRONT + DW + FRONT

# slot s holds taps (i=0..2, j=SLOT_J[s], k=SLOT_K[s]); slots 0-2 are
# DVE-fused fp8 multiplies (1x), slots 3-8 are ScalarE-upconverted (2x)
SLOT_J = [2, 0, 1, 2, 0, 1, 2, 0, 1]
SLOT_K = [1, 2, 2, 2, 0, 0, 0, 1, 1]
NFUSED = 3
NUP = 9 - NFUSED

F32 = mybir.dt.float32
IO_DT = mybir.dt.bfloat16
F8 = mybir.dt.float8e3


def _overlap_ap(tile_ap, start, dims):
    """AP on tile_ap's tensor at element offset `start` with custom free dims
    [[stride, num], ...] (keeps the tile's partition dim)."""
    return bass_rust.AP(tile_ap.tensor, start, [list(tile_ap.ap[0])] + dims)


def _build():
    nc = bacc.Bacc()
    x_ext = nc.declare_dram_parameter("input", [PAIRS, 128, XPLEN], IO_DT, isOutput=False)
    f_ext = nc.declare_dram_parameter("F", [PAIRS, 128, NCHUNK * CFD], F8, isOutput=False)
    s_ext = nc.declare_dram_parameter("stat", [128, 3 * 128], IO_DT, isOutput=False)
    out_ext = nc.declare_dram_parameter("out", [PAIRS, 128, NCHUNK * FD], IO_DT, isOutput=True)

    with tile.TileContext(nc) as tc:
        with (
            tc.tile_pool(name="const", bufs=1) as cpool,
            tc.tile_pool(name="xp", bufs=2) as xpool,
            tc.tile_pool(name="fp", bufs=3) as fpool,
            tc.tile_pool(name="fb", bufs=2) as fbpool,
            tc.tile_pool(name="prod", bufs=3) as ppool,
            tc.tile_pool(name="osb", bufs=3) as opool,
            tc.tile_pool(name="ps", bufs=4, space="PSUM") as pspool,
        ):
            st = cpool.tile([128, 3 * 128], IO_DT)

            # x/x2/st on the scalar ring; F on the sync ring; out D2Ds on
            # the gpsimd ring. Pair-0 x/x2 split so chunk 0's window (first
            # ~650 elems) lands fast.
            XCUT = 1152
            xps, x2s = [], []
            for pair in range(PAIRS):
                xp = xpool.tile([128, XPLEN], IO_DT, tag="xp")
                x2 = xpool.tile([128, XPLEN - 1], IO_DT, tag="x2")
                xps.append(xp)
                x2s.append(x2)
            # only what chunk 0 needs goes first; the rest of pair-0 x and
            # all of pair-1 x are issued later (needed at it>=2 / it>=4),
            # freeing early DMA bandwidth for the first F chunks
            nc.scalar.dma_start(xps[0][:, :XCUT], x_ext[0, :, :XCUT])
            nc.scalar.dma_start(x2s[0][:, :XCUT], x_ext[0, :, 1 : XCUT + 1])
            nc.scalar.dma_start(st[:], s_ext[:])

            for it in range(PAIRS * NCHUNK):
                if it == 1:
                    nc.scalar.dma_start(xps[0][:, XCUT:], x_ext[0, :, XCUT:])
                    nc.scalar.dma_start(
                        x2s[0][:, XCUT:], x_ext[0, :, XCUT + 1 : XPLEN]
                    )
                elif it == 2:
                    nc.scalar.dma_start(xps[1][:, :], x_ext[1, :, :])
                    nc.scalar.dma_start(x2s[1][:, :], x_ext[1, :, 1:XPLEN])
                pair, ch = divmod(it, NCHUNK)
                xp, x2 = xps[pair], x2s[pair]
                ft = fpool.tile([128, CFD], F8, tag="ft")
                base = ch * CFD
                first = it == 0
                last = it == PAIRS * NCHUNK - 1
                FB = NFUSED * SFD  # fused bytes/elems boundary
                if first:
                    # stream chunk 0 at sub-slot granularity for fast fill
                    nc.sync.dma_start(ft[:, :SFD], f_ext[pair, :, base : base + SFD])
                    nc.sync.dma_start(
                        ft[:, SFD:FB], f_ext[pair, :, base + SFD : base + FB]
                    )
                    mid = (NFUSED + 3) * SFD
                    nc.sync.dma_start(
                        ft[:, FB:mid], f_ext[pair, :, base + FB : base + mid]
                    )
                    nc.sync.dma_start(ft[:, mid:], f_ext[pair, :, base + mid : base + CFD])
                else:
                    nc.sync.dma_start(ft[:, :FB], f_ext[pair, :, base : base + FB])
                    nc.sync.dma_start(
                        ft[:, FB:], f_ext[pair, :, base + FB : base + CFD]
                    )

                fb = fbpool.tile([128, NUP * SFD], IO_DT, tag="fb")
                if first:
                    nc.scalar.copy(fb[:, : 3 * SFD], ft[:, FB : FB + 3 * SFD])
                    nc.scalar.copy(fb[:, 3 * SFD :], ft[:, FB + 3 * SFD :])
                else:
                    nc.scalar.copy(fb[:, :], ft[:, FB:])

                prod = ppool.tile([128, CFD], IO_DT, tag="prod")
                psum = pspool.tile([128, FD], F32, tag="ps")
                xb = ch * FD  # even window base; +k for k in {0,2} on xp, x2 for k=1

                def slot_aps(t, s0, n):
                    return t[:, s0 * SFD : (s0 + n) * SFD].rearrange(
                        "p (s i e) -> p s i e", s=n, i=3
                    )

                def slot_ap2(t, s):
                    return t[:, s * SFD : (s + 1) * SFD].rearrange(
                        "p (i e) -> p i e", i=3
                    )

                def mm(s, i, start=False, stop=False):
                    t = 3 * s + i
                    nc.tensor.matmul(
                        psum[:],
                        st[:, SLOT_J[s] * 128 : (SLOT_J[s] + 1) * 128],
                        prod[:, t * FD : (t + 1) * FD],
                        start=start,
                        stop=stop,
                    )

                def fused_muls():
                    # slot0 (k=1, via x2) alone; slots1-2 (k=2) merged
                    nc.vector.tensor_mul(
                        slot_ap2(prod, 0),
                        _overlap_ap(x2[:], xb, [[W, 3], [1, FD]]),
                        slot_ap2(ft, 0),
                    )
                    nc.vector.tensor_mul(
                        slot_aps(prod, 1, 2),
                        _overlap_ap(xp[:], xb + 2, [[0, 2], [W, 3], [1, FD]]),
                        slot_aps(ft, 1, 2),
                    )

                def up_muls():
                    # slot3 (k=2); slots4-6 (k=0) merged; slots7-8 (k=1 via
                    # x2) merged — all bf16 2x (even bases)
                    nc.vector.tensor_mul(
                        slot_ap2(prod, 3),
                        _overlap_ap(xp[:], xb + 2, [[W, 3], [1, FD]]),
                        fb[:, :SFD].rearrange("p (i e) -> p i e", i=3),
                    )
                    nc.vector.tensor_mul(
                        slot_aps(prod, 4, 3),
                        _overlap_ap(xp[:], xb, [[0, 3], [W, 3], [1, FD]]),
                        fb[:, SFD : 4 * SFD].rearrange(
                            "p (s i e) -> p s i e", s=3, i=3
                        ),
                    )
                    nc.vector.tensor_mul(
                        slot_aps(prod, 7, 2),
                        _overlap_ap(x2[:], xb, [[0, 2], [W, 3], [1, FD]]),
                        fb[:, 4 * SFD :].rearrange("p (s i e) -> p s i e", s=2, i=3),
                    )

                if not last:
                    fused_muls()
                    for s in range(NFUSED):
                        mm(s, 0, start=(s == 0))
                        mm(s, 1)
                        mm(s, 2)
                    up_muls()
                    for s in range(NFUSED, 9):
                        mm(s, 0)
                        mm(s, 1)
                        mm(s, 2, stop=(s == 8))
                else:
                    # tail: upconverted first, cheap fused instrs last so
                    # the final mms depend on early-arriving data
                    up_muls()
                    for s in range(NFUSED, 9):
                        mm(s, 0, start=(s == NFUSED))
                        mm(s, 1)
                        mm(s, 2)
                    fused_muls()
                    for s in range(NFUSED):
                        mm(s, 0)
                        mm(s, 1)
                        mm(s, 2, stop=(s == NFUSED - 1))
                osb = opool.tile([128, FD], IO_DT, tag="osb")
                if last:
                    # DVE is idle by the tail — don't queue the final evict
                    # behind the scalar ring
                    nc.vector.tensor_copy(osb[:], psum[:])
                else:
                    nc.scalar.copy(osb[:], psum[:])
                nc.gpsimd.dma_start(
                    out_ext[pair, :, ch * FD : (ch + 1) * FD], osb[:]
                )
    nc.compile()
    return nc


_NC_CACHE = {}


def _host_inputs(input, F):
    """FULL inputs -> per-core in_maps with the kernel's layouts."""
    io_np = mybir.dt.np(IO_DT)
    f8_np = mybir.dt.np(F8)
    # x dense rows: xs[bc, y, FRONT + z*W + x]
    xs = np.zeros((BC, H, XPLEN), dtype=io_np)
    xs[:, :, FRONT : FRONT + DW] = (
        input.reshape(BC, D, H, W).transpose(0, 2, 1, 3).reshape(BC, H, DW).astype(io_np)
    )
    xs = xs.reshape(BC // 2, 128, XPLEN)

    # F pre-shifted along y by -dy per j, slot-ordered taps, edge taps zeroed
    base = np.ascontiguousarray(
        F.reshape(BC, D, H, W, 3, 3, 3).transpose(0, 2, 5, 4, 6, 1, 3)
    )  # [bc, y, j, i, k, z, x]
    Hs = np.zeros_like(base)
    Hs[:, : H - 1, 0] = base[:, 1:, 0]
    Hs[:, :, 1] = base[:, :, 1]
    Hs[:, 1:, 2] = base[:, : H - 1, 2]
    Hs[:, :, :, :, 0, :, 0] = 0
    Hs[:, :, :, :, 2, :, W - 1] = 0
    Hs[:, :, :, 0, :, 0, :] = 0
    Hs[:, :, :, 2, :, D - 1, :] = 0
    # slot-major: [bc, y, s, i, z, x]
    Hs = np.stack([Hs[:, :, SLOT_J[s], :, SLOT_K[s]] for s in range(9)], axis=2)
    fs = (
        Hs.reshape(BC, H, 9, 3, NCHUNK, ZC, W)
        .transpose(0, 1, 4, 2, 3, 5, 6)  # [bc, y, ch, s, i, zc, x]
        .reshape(BC // 2, 128, NCHUNK * CFD)
        .astype(f8_np)
    )

    # stationaries: st[kk, j*128+m] = 1 iff kk == m + (j-1), same 64-block
    stm = np.zeros((128, 3, 128), dtype=np.float32)
    for j in range(3):
        Sj = np.eye(128, k=-(j - 1), dtype=np.float32)
        Sj[0:64, 64:128] = 0
        Sj[64:128, 0:64] = 0
        stm[:, j, :] = Sj
    stm = stm.reshape(128, 3 * 128).astype(io_np)

    return [
        {
            "input": xs[c * PAIRS : (c + 1) * PAIRS],
            "F": fs[c * PAIRS : (c + 1) * PAIRS],
            "stat": stm,
        }
        for c in range(N_CORES)
    ]


def kernel(input: np.ndarray, F: np.ndarray) -> np.ndarray:
    input = np.asarray(input)
    F = np.asarray(F)
    assert input.shape == (B, C, D, H, W), input.shape
    assert F.shape == (B, C, D, H, W, 3, 3, 3), F.shape

    if "nc" not in _NC_CACHE:
        _NC_CACHE["nc"] = _build()
    nc = _NC_CACHE["nc"]

    in_maps = _host_inputs(input, F)
    # the fleet occasionally throws transient NRT_EXEC_UNIT_UNRECOVERABLE
    # device errors (observed in dev, cleared on retry)
    last_err = None
    out = None
    for _attempt in range(4):
        try:
            res = run_bass_kernel_spmd(nc, in_maps, core_ids=list(range(N_CORES)))
        except Exception as e:  # noqa: BLE001
            last_err = e
            time.sleep(2.0)
            continue
        out = np.concatenate(
            [
                np.asarray(res.results[c]["out"], dtype=np.float32)
                for c in range(N_CORES)
            ],
            axis=0,
        )  # [BC/2, 128, NCHUNK*FD]
        if np.isfinite(out).all():
            break
        last_err = RuntimeError("non-finite output (transient device flake)")
    else:
        raise last_err
    out = (
        out.reshape(BC // 2, 2, H, NCHUNK, ZC, W)
        .transpose(0, 1, 3, 4, 2, 5)  # [pair, s, ch, zc, y, x]
        .reshape(B, C, D, H, W)
        .astype(np.float32)
    )
    return np.ascontiguousarray(out)


# revision 26
# speedup vs baseline: 1.0452x; 1.0452x over previous
"""nn_AdapFilter3d Trainium2 kernel — 8-core SPMD (data-parallel over (B,C)).

out[b,c,z,y,x] = sum_{i,j,k} pad(input)[b,c,z+i-1,y+j-1,x+k-1] * F[b,c,z,y,x,i,j,k]

v6 strategy (per NeuronCore: 4 of the 32 (b,c) slices = 2 slice-pairs;
partitions p = 64*s + y; free dims carry (z, x) densely):

  - F streams from HBM as fp8 E3M4 (4 mantissa bits): halves the dominant
    HBM stream (28.3MB -> 14.2MB/core). End-to-end rel err 1.28e-2 < 2e-2.
  - y-shift via accumulating matmuls with shift stationaries S_j (host
    pre-shifts F by -dy per j); x/z shifts are free-dim offsets into one
    padded dense (z,x) row per y-partition.
  - Taps in 9 slots (3 i-taps each; j,k fixed per slot). 3 slots multiply
    fused fp8 x bf16 on DVE (1x mode, 557ns/tap), 6 slots are ScalarE-
    upconverted fp8->bf16 (427ns/tap) then DVE bf16 2x (278ns/tap).
    Keeps Scalar duty ~60% — pushing more work onto ScalarE was measured
    to trigger a chip-wide ~20% downclock (P0/activity throttle).
  - Merged multi-slot DVE instrs (stride-0 j-broadcast APs) hit 2x fine;
    k=1 windows have odd bases, served via a +1-shifted second x copy.
  - gpsimd computes NOTHING (measured 6.8x DVE slowdown from SBUF-fabric
    contention); it only carries the output-DMA descriptors.
  - PE: 27 accumulating 512-col matmuls/chunk (216ns warm / 427 cold).

Self-contained: hardcodes shapes from the problem spec.
"""

import time

import numpy as np

import bass_rust
import concourse.bacc as bacc
import concourse.tile as tile
from concourse import mybir
from concourse.bass_utils import run_bass_kernel_spmd

B, C, D, H, W = 2, 16, 32, 64, 64
BC = B * C
TAPS = 27
N_CORES = 8
S = BC // N_CORES  # 4 slices per core
PAIRS = S // 2  # 2
ZC = 8  # z planes per chunk
NCHUNK = D // ZC  # 4
FD = ZC * W  # 512
SFD = 3 * FD  # 1536 (one slot = 3 i-taps)
CFD = TAPS * FD  # 13824 (one chunk of F)
DW = D * W  # 2048 dense (z,x) elements per (slice, y)
FRONT = 65  # zero pad around the dense (z,x) block (>= W+1)
XPLEN = FRONT + DW + FRONT

# slot s holds taps (i=0..2, j=SLOT_J[s], k=SLOT_K[s]); slots 0-2 are
# DVE-fused fp8 multiplies (1x), slots 3-8 are ScalarE-upconverted (2x)
SLOT_J = [2, 0, 1, 2, 0, 1, 2, 0, 1]
SLOT_K = [1, 2, 2, 2, 0, 0, 0, 1, 1]
NFUSED = 3
NUP = 9 - NFUSED

F32 = mybir.dt.float32
IO_DT = mybir.dt.bfloat16
F8 = mybir.dt.float8e3


def _overlap_ap(tile_ap, start, dims):
    """AP on tile_ap's tensor at element offset `start` with custom free dims
    [[stride, num], ...] (keeps the tile's partition dim)."""
    return bass_rust.AP(tile_ap.tensor, start, [list(tile_ap.ap[0])] + dims)


def _build():
    nc = bacc.Bacc()
    x_ext = nc.declare_dram_parameter("input", [PAIRS, 128, XPLEN], IO_DT, isOutput=False)
    f_ext = nc.declare_dram_parameter("F", [PAIRS, 128, NCHUNK * CFD], F8, isOutput=False)
    s_ext = nc.declare_dram_parameter("stat", [128, 3 * 128], IO_DT, isOutput=False)
    out_ext = nc.declare_dram_parameter("out", [PAIRS, 128, NCHUNK * FD], IO_DT, isOutput=True)

    with tile.TileContext(nc) as tc:
        with (
            tc.tile_pool(name="const", bufs=1) as cpool,
            tc.tile_pool(name="xp", bufs=2) as xpool,
            tc.tile_pool(name="fp", bufs=3) as fpool,
            tc.tile_pool(name="fb", bufs=2) as fbpool,
            tc.tile_pool(name="prod", bufs=3) as ppool,
            tc.tile_pool(name="osb", bufs=3) as opool,
            tc.tile_pool(name="ps", bufs=4, space="PSUM") as pspool,
        ):
            st = cpool.tile([128, 3 * 128], IO_DT)

            # x/x2/st on the scalar ring; F on the sync ring; out D2Ds on
            # the gpsimd ring. Pair-0 x/x2 split so chunk 0's window (first
            # ~650 elems) lands fast.
            XCUT = 1152
            xps, x2s = [], []
            for pair in range(PAIRS):
                xp = xpool.tile([128, XPLEN], IO_DT, tag="xp")
                x2 = xpool.tile([128, XPLEN - 1], IO_DT, tag="x2")
                if pair == 0:
                    nc.scalar.dma_start(xp[:, :XCUT], x_ext[pair, :, :XCUT])
                    nc.scalar.dma_start(x2[:, :XCUT], x_ext[pair, :, 1 : XCUT + 1])
                    nc.scalar.dma_start(st[:], s_ext[:])
                    nc.scalar.dma_start(xp[:, XCUT:], x_ext[pair, :, XCUT:])
                    nc.scalar.dma_start(
                        x2[:, XCUT:], x_ext[pair, :, XCUT + 1 : XPLEN]
                    )
                else:
                    nc.scalar.dma_start(xp[:, :], x_ext[pair, :, :])
                    nc.scalar.dma_start(x2[:, :], x_ext[pair, :, 1:XPLEN])
                xps.append(xp)
                x2s.append(x2)

            for it in range(PAIRS * NCHUNK):
                pair, ch = divmod(it, NCHUNK)
                xp, x2 = xps[pair], x2s[pair]
                ft = fpool.tile([128, CFD], F8, tag="ft")
                base = ch * CFD
                first = it == 0
                last = it == PAIRS * NCHUNK - 1
                FB = NFUSED * SFD  # fused bytes/elems boundary
                if first:
                    # stream chunk 0 at sub-slot granularity for fast fill
                    nc.sync.dma_start(ft[:, :SFD], f_ext[pair, :, base : base + SFD])
                    nc.sync.dma_start(
                        ft[:, SFD:FB], f_ext[pair, :, base + SFD : base + FB]
                    )
                    mid = (NFUSED + 3) * SFD
                    nc.sync.dma_start(
                        ft[:, FB:mid], f_ext[pair, :, base + FB : base + mid]
                    )
                    nc.sync.dma_start(ft[:, mid:], f_ext[pair, :, base + mid : base + CFD])
                else:
                    nc.sync.dma_start(ft[:, :FB], f_ext[pair, :, base : base + FB])
                    nc.sync.dma_start(
                        ft[:, FB:], f_ext[pair, :, base + FB : base + CFD]
                    )

                fb = fbpool.tile([128, NUP * SFD], IO_DT, tag="fb")
                if first:
                    nc.scalar.copy(fb[:, : 3 * SFD], ft[:, FB : FB + 3 * SFD])
                    nc.scalar.copy(fb[:, 3 * SFD :], ft[:, FB + 3 * SFD :])
                else:
                    nc.scalar.copy(fb[:, :], ft[:, FB:])

                prod = ppool.tile([128, CFD], IO_DT, tag="prod")
                psum = pspool.tile([128, FD], F32, tag="ps")
                xb = ch * FD  # even window base; +k for k in {0,2} on xp, x2 for k=1

                def slot_aps(t, s0, n):
                    return t[:, s0 * SFD : (s0 + n) * SFD].rearrange(
                        "p (s i e) -> p s i e", s=n, i=3
                    )

                def slot_ap2(t, s):
                    return t[:, s * SFD : (s + 1) * SFD].rearrange(
                        "p (i e) -> p i e", i=3
                    )

                def mm(s, i, start=False, stop=False):
                    t = 3 * s + i
                    nc.tensor.matmul(
                        psum[:],
                        st[:, SLOT_J[s] * 128 : (SLOT_J[s] + 1) * 128],
                        prod[:, t * FD : (t + 1) * FD],
                        start=start,
                        stop=stop,
                    )

                def fused_muls():
                    # slot0 (k=1, via x2) alone; slots1-2 (k=2) merged
                    nc.vector.tensor_mul(
                        slot_ap2(prod, 0),
                        _overlap_ap(x2[:], xb, [[W, 3], [1, FD]]),
                        slot_ap2(ft, 0),
                    )
                    nc.vector.tensor_mul(
                        slot_aps(prod, 1, 2),
                        _overlap_ap(xp[:], xb + 2, [[0, 2], [W, 3], [1, FD]]),
                        slot_aps(ft, 1, 2),
                    )

                def up_muls():
                    # slot3 (k=2); slots4-6 (k=0) merged; slots7-8 (k=1 via
                    # x2) merged — all bf16 2x (even bases)
                    nc.vector.tensor_mul(
                        slot_ap2(prod, 3),
                        _overlap_ap(xp[:], xb + 2, [[W, 3], [1, FD]]),
                        fb[:, :SFD].rearrange("p (i e) -> p i e", i=3),
                    )
                    nc.vector.tensor_mul(
                        slot_aps(prod, 4, 3),
                        _overlap_ap(xp[:], xb, [[0, 3], [W, 3], [1, FD]]),
                        fb[:, SFD : 4 * SFD].rearrange(
                            "p (s i e) -> p s i e", s=3, i=3
                        ),
                    )
                    nc.vector.tensor_mul(
                        slot_aps(prod, 7, 2),
                        _overlap_ap(x2[:], xb, [[0, 2], [W, 3], [1, FD]]),
                        fb[:, 4 * SFD :].rearrange("p (s i e) -> p s i e", s=2, i=3),
                    )

                if not last:
                    fused_muls()
                    for s in range(NFUSED):
                        mm(s, 0, start=(s == 0))
                        mm(s, 1)
                        mm(s, 2)
                    up_muls()
                    for s in range(NFUSED, 9):
                        mm(s, 0)
                        mm(s, 1)
                        mm(s, 2, stop=(s == 8))
                else:
                    # tail: upconverted first, cheap fused instrs last so
                    # the final mms depend on early-arriving data
                    up_muls()
                    for s in range(NFUSED, 9):
                        mm(s, 0, start=(s == NFUSED))
                        mm(s, 1)
                        mm(s, 2)
                    fused_muls()
                    for s in range(NFUSED):
                        mm(s, 0)
                        mm(s, 1)
                        mm(s, 2, stop=(s == NFUSED - 1))
                osb = opool.tile([128, FD], IO_DT, tag="osb")
                nc.scalar.copy(osb[:], psum[:])
                nc.gpsimd.dma_start(
                    out_ext[pair, :, ch * FD : (ch + 1) * FD], osb[:]
                )
    nc.compile()
    return nc


_NC_CACHE = {}


def _host_inputs(input, F):
    """FULL inputs -> per-core in_maps with the kernel's layouts."""
    io_np = mybir.dt.np(IO_DT)
    f8_np = mybir.dt.np(F8)
    # x dense rows: xs[bc, y, FRONT + z*W + x]
    xs = np.zeros((BC, H, XPLEN), dtype=io_np)
    xs[:, :, FRONT : FRONT + DW] = (
        input.reshape(BC, D, H, W).transpose(0, 2, 1, 3).reshape(BC, H, DW).astype(io_np)
    )
    xs = xs.reshape(BC // 2, 128, XPLEN)

    # F pre-shifted along y by -dy per j, slot-ordered taps, edge taps zeroed
    base = np.ascontiguousarray(
        F.reshape(BC, D, H, W, 3, 3, 3).transpose(0, 2, 5, 4, 6, 1, 3)
    )  # [bc, y, j, i, k, z, x]
    Hs = np.zeros_like(base)
    Hs[:, : H - 1, 0] = base[:, 1:, 0]
    Hs[:, :, 1] = base[:, :, 1]
    Hs[:, 1:, 2] = base[:, : H - 1, 2]
    Hs[:, :, :, :, 0, :, 0] = 0
    Hs[:, :, :, :, 2, :, W - 1] = 0
    Hs[:, :, :, 0, :, 0, :] = 0
    Hs[:, :, :, 2, :, D - 1, :] = 0
    # slot-major: [bc, y, s, i, z, x]
    Hs = np.stack([Hs[:, :, SLOT_J[s], :, SLOT_K[s]] for s in range(9)], axis=2)
    fs = (
        Hs.reshape(BC, H, 9, 3, NCHUNK, ZC, W)
        .transpose(0, 1, 4, 2, 3, 5, 6)  # [bc, y, ch, s, i, zc, x]
        .reshape(BC // 2, 128, NCHUNK * CFD)
        .astype(f8_np)
    )

    # stationaries: st[kk, j*128+m] = 1 iff kk == m + (j-1), same 64-block
    stm = np.zeros((128, 3, 128), dtype=np.float32)
    for j in range(3):
        Sj = np.eye(128, k=-(j - 1), dtype=np.float32)
        Sj[0:64, 64:128] = 0
        Sj[64:128, 0:64] = 0
        stm[:, j, :] = Sj
    stm = stm.reshape(128, 3 * 128).astype(io_np)

    return [
        {
            "input": xs[c * PAIRS : (c + 1) * PAIRS],
            "F": fs[c * PAIRS : (c + 1) * PAIRS],
            "stat": stm,
        }
        for c in range(N_CORES)
    ]


def kernel(input: np.ndarray, F: np.ndarray) -> np.ndarray:
    input = np.asarray(input)
    F = np.asarray(F)
    assert input.shape == (B, C, D, H, W), input.shape
    assert F.shape == (B, C, D, H, W, 3, 3, 3), F.shape

    if "nc" not in _NC_CACHE:
        _NC_CACHE["nc"] = _build()
    nc = _NC_CACHE["nc"]

    in_maps = _host_inputs(input, F)
    # the fleet occasionally throws transient NRT_EXEC_UNIT_UNRECOVERABLE
    # device errors (observed in dev, cleared on retry)
    last_err = None
    out = None
    for _attempt in range(4):
        try:
            res = run_bass_kernel_spmd(nc, in_maps, core_ids=list(range(N_CORES)))
        except Exception as e:  # noqa: BLE001
            last_err = e
            time.sleep(2.0)
            continue
        out = np.concatenate(
            [
                np.asarray(res.results[c]["out"], dtype=np.float32)
                for c in range(N_CORES)
            ],
            axis=0,
        )  # [BC/2, 128, NCHUNK*FD]
        if np.isfinite(out).all():
            break
        last_err = RuntimeError("non-finite output (transient device flake)")
    else:
        raise last_err
    out = (
        out.reshape(BC // 2, 2, H, NCHUNK, ZC, W)
        .transpose(0, 1, 3, 4, 2, 5)  # [pair, s, ch, zc, y, x]
        .reshape(B, C, D, H, W)
        .astype(np.float32)
    )
    return np.ascontiguousarray(out)
